# revision 1
# baseline (speedup 1.0000x reference)
"""Distributed scatter-max (segment max over edge targets) on 8 TRN2 NeuronCores.

Strategy (per the segment-parallel sharding hint):
  * Host shuffles edges: sort by target node, assign contiguous node-id
    ranges to 8 cores x 4 lanes (balanced by padded edge count), pad each
    node's edge run to a multiple of W=8 so no pooling window straddles
    two nodes, and lay each lane out feature-major (D=32 dims on SBUF
    partitions, edges along the free axis).
  * Device (same SPMD graph on all 8 cores) streams its (NT, 128, T)
    slab: window-W max-pool along the free axis (vector tensor_reduce),
    then a segmented running max via tensor_tensor_scan
    (state = max(state + reset_mask, block_max); reset_mask is 0 inside a
    node's run and -1e30 at run starts).
  * Host gathers the per-node result from each node's last block column
    and scatters back to node order; empty nodes stay 0.
"""
import sys

import numpy as np

try:
    import concourse.bacc as bacc
except ImportError:
    sys.path.insert(0, "/opt/trn_rl_repo")
    import concourse.bacc as bacc

import concourse.tile as tile
from concourse import mybir
from concourse.bass_utils import run_bass_kernel_spmd

C = 8            # cores
L = 4            # lanes per core (L * D = 128 partitions)
D = 32           # feature dim
W = 8            # pool window; node runs padded to multiples of W
T = 4096         # edge slots per lane per tile
P = 128
NBT = T // W
FILL = -1.0e30

_graph_cache = {}


def _build_graph(NT):
    if NT in _graph_cache:
        return _graph_cache[NT]
    nc = bacc.Bacc()
    x_ext = nc.declare_dram_parameter("xt", [NT, P, T], mybir.dt.float32, isOutput=False)
    a_ext = nc.declare_dram_parameter("amask", [NT, P, NBT], mybir.dt.float32, isOutput=False)
    out_ext = nc.declare_dram_parameter("out", [NT, P, NBT], mybir.dt.float32, isOutput=True)
    with tile.TileContext(nc) as tc:
        with tc.tile_pool(name="x", bufs=3) as xp, \
             tc.tile_pool(name="a", bufs=3) as apool, \
             tc.tile_pool(name="bm", bufs=3) as bpool, \
             tc.tile_pool(name="sm", bufs=3) as spool:
            for i in range(NT):
                xt = xp.tile([P, T], mybir.dt.float32)
                nc.sync.dma_start(out=xt[:], in_=x_ext[i, :, :])
                at = apool.tile([P, NBT], mybir.dt.float32)
                nc.sync.dma_start(out=at[:], in_=a_ext[i, :, :])
                bm = bpool.tile([P, NBT], mybir.dt.float32)
                nc.vector.tensor_reduce(
                    out=bm[:],
                    in_=xt[:].rearrange("p (nb w) -> p nb w", w=W),
                    axis=mybir.AxisListType.X,
                    op=mybir.AluOpType.max,
                )
                sm = spool.tile([P, NBT], mybir.dt.float32)
                nc.vector.tensor_tensor_scan(
                    out=sm[:], data0=at[:], data1=bm[:], initial=FILL,
                    op0=mybir.AluOpType.add, op1=mybir.AluOpType.max,
                )
                nc.sync.dma_start(out=out_ext[i, :, :], in_=sm[:])
    nc.finalize()
    _graph_cache[NT] = nc
    return nc


def _layout(tgt, N):
    """Plan the edge shuffle. Returns placement metadata (all vectorized)."""
    E = tgt.shape[0]
    deg = np.bincount(tgt, minlength=N)
    pd = ((deg + W - 1) // W) * W            # per-node padded slot count

    # contiguous node ranges per (core, lane), balanced by padded count
    cum = np.cumsum(pd)
    total = int(cum[-1])
    nlanes = C * L
    marks = [total * k // nlanes for k in range(1, nlanes)]
    b = np.searchsorted(cum, marks, side="left")
    bounds = np.concatenate([[0], b + 1, [N]])
    bounds = np.maximum.accumulate(np.minimum(bounds, N))

    node_lane = np.full(N, -1, dtype=np.int32)   # global lane id 0..31
    node_pos = np.zeros(N, dtype=np.int64)       # start slot within lane
    lane_tiles = np.zeros(nlanes, dtype=np.int64)

    for g in range(nlanes):
        lo, hi = int(bounds[g]), int(bounds[g + 1])
        ids = np.flatnonzero(pd[lo:hi] > 0) + lo
        if ids.size == 0:
            continue
        s = pd[ids]
        csum = np.concatenate([[0], np.cumsum(s)])   # (n+1,)
        pos = np.empty(ids.size, dtype=np.int64)
        base = 0
        t = 0
        n = ids.size
        while base < n:
            k = int(np.searchsorted(csum, csum[base] + T, side="right")) - 1
            pos[base:k] = t * T + (csum[base:k] - csum[base])
            t += 1
            base = k
        node_lane[ids] = g
        node_pos[ids] = pos
        lane_tiles[g] = t

    NT = int(lane_tiles.max())
    Q = NT * T
    NB = Q // W

    # per-edge slot assignment
    order = np.argsort(tgt, kind="stable")
    sorted_tgt = tgt[order]
    starts = np.searchsorted(sorted_tgt, np.arange(N), side="left")
    rank = np.arange(E, dtype=np.int64) - starts[sorted_tgt]
    slot = node_pos[sorted_tgt] + rank
    elane = node_lane[sorted_tgt]

    # scan reset mask per lane: FILL everywhere except continuation blocks
    nz = pd > 0
    amask = np.full((nlanes, NB), FILL, dtype=np.float32)
    bstart = (node_pos[nz] // W).astype(np.int64)
    bend = ((node_pos[nz] + pd[nz]) // W).astype(np.int64)
    lanes_nz = node_lane[nz].astype(np.int64)
    mark = np.zeros(nlanes * (NB + 1), dtype=np.int32)
    np.add.at(mark, lanes_nz * (NB + 1) + bstart + 1, 1)
    np.add.at(mark, lanes_nz * (NB + 1) + bend, -1)
    cont = np.cumsum(mark.reshape(nlanes, NB + 1)[:, :-1], axis=1) > 0
    amask[cont] = 0.0

    last_block = np.where(nz, (node_pos + pd) // W - 1, -1)
    return dict(order=order, slot=slot, elane=elane, NT=NT, NB=NB,
                amask=amask, node_lane=node_lane, last_block=last_block, nz=nz)


def kernel(source_node_representation_with_coefficient, edge_index, num_nodes):
    x = np.ascontiguousarray(np.asarray(source_node_representation_with_coefficient,
                                        dtype=np.float32))
    tgt = np.asarray(edge_index)[1].astype(np.int64)
    N = int(num_nodes)
    E, d = x.shape
    assert d == D

    ly = _layout(tgt, N)
    NT, NB = ly["NT"], ly["NB"]
    Q = NT * T

    # scatter edge features into the padded lane layout, one big gather
    perm = np.full((C * L, Q), E, dtype=np.int64)
    perm[ly["elane"], ly["slot"]] = ly["order"]
    x_aug = np.concatenate([x, np.full((1, D), FILL, dtype=np.float32)], axis=0)
    g = x_aug[perm]                                  # (32, Q, D)
    g = g.reshape(C, L, NT, T, D).transpose(0, 2, 1, 4, 3)
    xt_all = np.ascontiguousarray(g.reshape(C, NT, P, T))

    am = ly["amask"].reshape(C, L, NT, NBT)
    am = np.broadcast_to(am[:, :, :, None, :], (C, L, NT, D, NBT))
    am = np.ascontiguousarray(am.transpose(0, 2, 1, 3, 4).reshape(C, NT, P, NBT))

    in_maps = [{"xt": xt_all[c], "amask": am[c]} for c in range(C)]
    nc = _build_graph(NT)
    res = run_bass_kernel_spmd(nc, in_maps, core_ids=list(range(C)))

    # host gather: value for node n, dim d0 = v[core, 32*lane_in_core + d0, last_block]
    v = np.stack([r["out"].transpose(1, 0, 2).reshape(P, NB) for r in res.results])
    out = np.zeros((N, D), dtype=np.float32)
    nzi = np.flatnonzero(ly["nz"])
    gl = ly["node_lane"][nzi].astype(np.int64)
    core = gl // L
    lane = gl % L
    lb = ly["last_block"][nzi]
    rows = (lane * D)[:, None] + np.arange(D)[None, :]
    out[nzi] = v[core[:, None], rows, lb[:, None]]
    return out


# revision 2
# speedup vs baseline: 1.9960x; 1.9960x over previous
"""Distributed scatter-max (segment max over edge targets) on 8 TRN2 NeuronCores.

Strategy (segment-parallel scatter per the sharding hint):
  * Host shuffles edges: sort by target node; group nodes by exact degree and
    deal each degree class round-robin into 32 streams (8 cores x 4 lanes), so
    every stream has the IDENTICAL sequence of (window, count) chunks -> one
    SPMD graph serves all cores. Each node's edges are contiguous in its
    stream; streams are laid out feature-major (D=32 dims on SBUF partitions,
    edge slots along the free axis), cast to fp16 (max is order-exact; fp16
    rounding keeps rel err ~3e-4, far under the 2e-2 gate).
  * Device (same graph on all 8 cores) streams its (128, Q) slab tile by tile
    on the sync HWDGE queue and runs one windowed max tensor_reduce per degree
    chunk (window = exact degree -> zero compute waste), writing per-node
    columns; per-tile output slices leave on the scalar HWDGE queue so output
    never blocks the input stream.
  * Host gathers each node's column, casts back to f32; empty nodes stay 0.
"""
import sys

import numpy as np

try:
    import concourse.bacc as bacc
except ImportError:
    sys.path.insert(0, "/opt/trn_rl_repo")
    import concourse.bacc as bacc

import concourse.tile as tile
from concourse import mybir
from concourse.bass_utils import run_bass_kernel_spmd

C = 8            # cores
L = 4            # lanes per core (L * D = 128 partitions)
D = 32           # feature dim
P = 128
NLANES = C * L
T_MAX = 4096     # max edge slots per tile
FILL = -60000.0  # pad value (below any fp16 data value we handle)
DT = mybir.dt.float16

_graph_cache = {}


def _plan(tgt, N):
    """Host-side shuffle plan. Returns layout dict (see kernel())."""
    E = tgt.shape[0]
    deg = np.bincount(tgt, minlength=N).astype(np.int64)

    nz = np.flatnonzero(deg > 0)
    order_by_deg = nz[np.argsort(deg[nz], kind="stable")]
    degs_sorted = deg[order_by_deg]
    uniq, counts = np.unique(degs_sorted, return_counts=True)

    node_lane = np.full(N, -1, dtype=np.int32)
    node_rank = np.full(N, -1, dtype=np.int64)   # per-lane sequence index
    classes = []          # (window s, nodes per lane)
    seq_len = 0
    base = 0
    for s, cnt in zip(uniq, counts):
        ids = order_by_deg[base:base + cnt]
        base += cnt
        npl = (cnt + NLANES - 1) // NLANES
        node_lane[ids] = np.arange(cnt, dtype=np.int32) % NLANES
        node_rank[ids] = seq_len + np.arange(cnt, dtype=np.int64) // NLANES
        classes.append((int(s), int(npl)))
        seq_len += npl

    needed0 = sum(s * npl for s, npl in classes)

    def walk(bounds):
        chunks = []           # (tile, off, n, s, col0)
        node_pos_seq = np.empty(seq_len, dtype=np.int64)
        ti = pos = col = seq_base = 0
        for s, npl in classes:
            remaining = npl
            while remaining > 0:
                if ti >= len(bounds):
                    return None
                start, size = bounds[ti]
                space = start + size - pos
                fit = min(remaining, space // s)
                if fit == 0:
                    pos = start + size
                    ti += 1
                    continue
                chunks.append((ti, int(pos - start), int(fit), int(s), int(col)))
                idx0 = seq_base + (npl - remaining)
                node_pos_seq[idx0:idx0 + fit] = pos + np.arange(fit, dtype=np.int64) * s
                pos += fit * s
                col += fit
                remaining -= fit
            seq_base += npl
        return chunks, node_pos_seq, col

    slack = 0
    while True:
        rem = needed0 + slack
        sizes = []
        while rem > T_MAX:
            sizes.append(T_MAX)
            rem -= T_MAX
        sizes.append(((rem + 63) // 64) * 64)
        bounds = []
        acc = 0
        for t in sizes:
            bounds.append((acc, t))
            acc += t
        r = walk(bounds)
        if r is not None:
            break
        slack += 256
    chunks, node_pos_seq, NN = r

    node_pos = np.zeros(N, dtype=np.int64)
    node_col = np.full(N, -1, dtype=np.int64)
    m = node_rank >= 0
    node_pos[m] = node_pos_seq[node_rank[m]]
    node_col[m] = node_rank[m]           # cols assigned in walk (= seq) order

    order = np.argsort(tgt, kind="stable")
    sorted_tgt = tgt[order]
    starts = np.searchsorted(sorted_tgt, np.arange(N), side="left")
    rank = np.arange(E, dtype=np.int64) - starts[sorted_tgt]
    slot = node_pos[sorted_tgt] + rank
    elane = node_lane[sorted_tgt]

    return dict(chunks=chunks, tile_bounds=bounds, Q=int(acc), NN=int(NN),
                node_lane=node_lane, node_col=node_col, nz=deg > 0,
                order=order, slot=slot, elane=elane)


def _build_graph(ly):
    key = (ly["Q"], ly["NN"], tuple(ly["tile_bounds"]), tuple(ly["chunks"]))
    if key in _graph_cache:
        return _graph_cache[key]
    Q, NN = ly["Q"], ly["NN"]
    tile_bounds = ly["tile_bounds"]
    NT = len(tile_bounds)
    nc = bacc.Bacc()
    x_ext = nc.declare_dram_parameter("xt", [P, Q], DT, isOutput=False)
    out_ext = nc.declare_dram_parameter("out", [P, NN], DT, isOutput=True)
    by_tile = [[] for _ in range(NT)]
    for (ti, off, n, s, col0) in ly["chunks"]:
        by_tile[ti].append((off, n, s, col0))
    tile_cols = []
    for i in range(NT):
        cs = by_tile[i]
        c0 = min(c[3] for c in cs) if cs else 0
        c1 = max(c[3] + c[1] for c in cs) if cs else 0
        tile_cols.append((c0, c1))

    with tile.TileContext(nc) as tc:
        with tc.tile_pool(name="x", bufs=3) as xp, \
             tc.tile_pool(name="o", bufs=3) as opool:
            for i in range(NT):
                start, size = tile_bounds[i]
                xt = xp.tile([P, size], DT, tag="xt")
                nc.sync.dma_start(out=xt[:], in_=x_ext[:, start:start + size])
                c0, c1 = tile_cols[i]
                if c1 <= c0:
                    continue
                ot = opool.tile([P, c1 - c0], DT, tag="ot")
                for (off, n, s, col0) in by_tile[i]:
                    if s == 1:
                        nc.vector.tensor_copy(ot[:, col0 - c0:col0 - c0 + n],
                                              xt[:, off:off + n])
                    else:
                        nc.vector.tensor_reduce(
                            out=ot[:, col0 - c0:col0 - c0 + n],
                            in_=xt[:, off:off + n * s].rearrange(
                                "p (n s) -> p n s", s=s),
                            axis=mybir.AxisListType.X,
                            op=mybir.AluOpType.max,
                        )
                nc.scalar.dma_start(out=out_ext[:, c0:c1], in_=ot[:])
    nc.finalize()
    _graph_cache[key] = nc
    return nc


def kernel(source_node_representation_with_coefficient, edge_index, num_nodes):
    x = np.asarray(source_node_representation_with_coefficient, dtype=np.float32)
    tgt = np.asarray(edge_index)[1].astype(np.int64)
    N = int(num_nodes)
    E, d = x.shape
    assert d == D, f"kernel hardcodes D={D}, got {d}"
    if E == 0 or N == 0:
        return np.zeros((N, D), dtype=np.float32)

    ly = _plan(tgt, N)
    Q = ly["Q"]

    # scatter fp16-cast edge features into the padded lane layout
    x16 = np.clip(x, -60000.0, 60000.0).astype(np.float16)
    perm = np.full((NLANES, Q), E, dtype=np.int64)
    perm[ly["elane"], ly["slot"]] = ly["order"]
    x_aug = np.concatenate(
        [x16, np.full((1, D), FILL, dtype=np.float16)], axis=0)
    g = x_aug[perm]                                   # (32, Q, D)
    g = g.reshape(C, L, Q, D).transpose(0, 1, 3, 2)   # (C, L, D, Q)
    xt_all = np.ascontiguousarray(g.reshape(C, P, Q))

    nc = _build_graph(ly)
    in_maps = [{"xt": xt_all[c]} for c in range(C)]
    res = run_bass_kernel_spmd(nc, in_maps, core_ids=list(range(C)))

    v = np.stack([res.results[c]["out"] for c in range(C)])   # (C, P, NN) f16
    out = np.zeros((N, D), dtype=np.float32)
    nzi = np.flatnonzero(ly["nz"])
    gl = ly["node_lane"][nzi].astype(np.int64)
    core, lane = gl // L, gl % L
    colv = ly["node_col"][nzi]
    rows = (lane * D)[:, None] + np.arange(D)[None, :]
    out[nzi] = v[core[:, None], rows, colv[:, None]].astype(np.float32)
    return out


# revision 4
# speedup vs baseline: 1.9979x; 1.0010x over previous
"""Distributed scatter-max (segment max over edge targets) on 8 TRN2 NeuronCores.

Strategy (segment-parallel scatter per the sharding hint):
  * Host shuffles edges: sort by target node; group nodes by exact degree and
    deal each degree class round-robin into 32 streams (8 cores x 4 lanes), so
    every stream has the IDENTICAL sequence of (window, count) chunks -> one
    SPMD graph serves all cores. Each node's edges are contiguous in its
    stream; streams are laid out feature-major (D=32 dims on SBUF partitions,
    edge slots along the free axis), cast to fp16 (max is order-exact; fp16
    rounding keeps rel err ~3e-4, far under the 2e-2 gate).
  * Device (same graph on all 8 cores) streams its (128, Q) slab tile by tile
    on the sync HWDGE queue and runs one windowed max tensor_reduce per degree
    chunk (window = exact degree -> zero compute waste), writing per-node
    columns; per-tile output slices leave on the scalar HWDGE queue so output
    never blocks the input stream.
  * Host gathers each node's column, casts back to f32; empty nodes stay 0.
"""
import sys

import numpy as np

try:
    import concourse.bacc as bacc
except ImportError:
    sys.path.insert(0, "/opt/trn_rl_repo")
    import concourse.bacc as bacc

import concourse.tile as tile
from concourse import mybir
from concourse.bass_utils import run_bass_kernel_spmd

C = 8            # cores
L = 4            # lanes per core (L * D = 128 partitions)
D = 32           # feature dim
P = 128
NLANES = C * L
T_MAX = 4096     # max edge slots per tile
FILL = -60000.0  # pad value (below any fp16 data value we handle)
DT = mybir.dt.float16

_graph_cache = {}


def _plan(tgt, N):
    """Host-side shuffle plan. Returns layout dict (see kernel())."""
    E = tgt.shape[0]
    deg = np.bincount(tgt, minlength=N).astype(np.int64)

    nz = np.flatnonzero(deg > 0)
    order_by_deg = nz[np.argsort(deg[nz], kind="stable")]
    degs_sorted = deg[order_by_deg]
    uniq, counts = np.unique(degs_sorted, return_counts=True)

    node_lane = np.full(N, -1, dtype=np.int32)
    node_rank = np.full(N, -1, dtype=np.int64)   # per-lane sequence index
    classes = []          # (window s, nodes per lane)
    seq_len = 0
    base = 0
    for s, cnt in zip(uniq, counts):
        ids = order_by_deg[base:base + cnt]
        base += cnt
        npl = (cnt + NLANES - 1) // NLANES
        node_lane[ids] = np.arange(cnt, dtype=np.int32) % NLANES
        node_rank[ids] = seq_len + np.arange(cnt, dtype=np.int64) // NLANES
        classes.append((int(s), int(npl)))
        seq_len += npl

    needed0 = sum(s * npl for s, npl in classes)
    # a node's window must fit inside one tile
    max_s = max(s for s, _ in classes)
    t_max = max(T_MAX, ((max_s + 63) // 64) * 64)
    assert max_s <= 24576, f"node degree {max_s} exceeds supported maximum"

    def walk(bounds):
        chunks = []           # (tile, off, n, s, col0)
        node_pos_seq = np.empty(seq_len, dtype=np.int64)
        ti = pos = col = seq_base = 0
        for s, npl in classes:
            remaining = npl
            while remaining > 0:
                if ti >= len(bounds):
                    return None
                start, size = bounds[ti]
                space = start + size - pos
                fit = min(remaining, space // s)
                if fit == 0:
                    pos = start + size
                    ti += 1
                    continue
                chunks.append((ti, int(pos - start), int(fit), int(s), int(col)))
                idx0 = seq_base + (npl - remaining)
                node_pos_seq[idx0:idx0 + fit] = pos + np.arange(fit, dtype=np.int64) * s
                pos += fit * s
                col += fit
                remaining -= fit
            seq_base += npl
        return chunks, node_pos_seq, col

    slack = 0
    while True:
        rem = needed0 + slack
        sizes = []
        while rem > t_max:
            sizes.append(t_max)
            rem -= t_max
        sizes.append(((rem + 63) // 64) * 64)
        bounds = []
        acc = 0
        for t in sizes:
            bounds.append((acc, t))
            acc += t
        r = walk(bounds)
        if r is not None:
            break
        slack += 256
    chunks, node_pos_seq, NN = r

    node_pos = np.zeros(N, dtype=np.int64)
    node_col = np.full(N, -1, dtype=np.int64)
    m = node_rank >= 0
    node_pos[m] = node_pos_seq[node_rank[m]]
    node_col[m] = node_rank[m]           # cols assigned in walk (= seq) order

    order = np.argsort(tgt, kind="stable")
    sorted_tgt = tgt[order]
    starts = np.searchsorted(sorted_tgt, np.arange(N), side="left")
    rank = np.arange(E, dtype=np.int64) - starts[sorted_tgt]
    slot = node_pos[sorted_tgt] + rank
    elane = node_lane[sorted_tgt]

    return dict(chunks=chunks, tile_bounds=bounds, Q=int(acc), NN=int(NN),
                node_lane=node_lane, node_col=node_col, nz=deg > 0,
                order=order, slot=slot, elane=elane)


def _build_graph(ly):
    key = (ly["Q"], ly["NN"], tuple(ly["tile_bounds"]), tuple(ly["chunks"]))
    if key in _graph_cache:
        return _graph_cache[key]
    Q, NN = ly["Q"], ly["NN"]
    tile_bounds = ly["tile_bounds"]
    NT = len(tile_bounds)
    nc = bacc.Bacc()
    x_ext = nc.declare_dram_parameter("xt", [P, Q], DT, isOutput=False)
    out_ext = nc.declare_dram_parameter("out", [P, NN], DT, isOutput=True)
    by_tile = [[] for _ in range(NT)]
    for (ti, off, n, s, col0) in ly["chunks"]:
        by_tile[ti].append((off, n, s, col0))
    tile_cols = []
    for i in range(NT):
        cs = by_tile[i]
        c0 = min(c[3] for c in cs) if cs else 0
        c1 = max(c[3] + c[1] for c in cs) if cs else 0
        tile_cols.append((c0, c1))

    with tile.TileContext(nc) as tc:
        with tc.tile_pool(name="x", bufs=3) as xp, \
             tc.tile_pool(name="o", bufs=3) as opool:
            for i in range(NT):
                start, size = tile_bounds[i]
                xt = xp.tile([P, size], DT, tag="xt")
                nc.sync.dma_start(out=xt[:], in_=x_ext[:, start:start + size])
                c0, c1 = tile_cols[i]
                if c1 <= c0:
                    continue
                ot = opool.tile([P, c1 - c0], DT, tag="ot")
                for (off, n, s, col0) in by_tile[i]:
                    if s == 1:
                        nc.vector.tensor_copy(ot[:, col0 - c0:col0 - c0 + n],
                                              xt[:, off:off + n])
                    else:
                        nc.vector.tensor_reduce(
                            out=ot[:, col0 - c0:col0 - c0 + n],
                            in_=xt[:, off:off + n * s].rearrange(
                                "p (n s) -> p n s", s=s),
                            axis=mybir.AxisListType.X,
                            op=mybir.AluOpType.max,
                        )
                nc.scalar.dma_start(out=out_ext[:, c0:c1], in_=ot[:])
    nc.finalize()
    _graph_cache[key] = nc
    return nc


def kernel(source_node_representation_with_coefficient, edge_index, num_nodes):
    x = np.asarray(source_node_representation_with_coefficient, dtype=np.float32)
    tgt = np.asarray(edge_index)[1].astype(np.int64)
    N = int(num_nodes)
    E, d = x.shape
    assert d == D, f"kernel hardcodes D={D}, got {d}"
    if E == 0 or N == 0:
        return np.zeros((N, D), dtype=np.float32)

    ly = _plan(tgt, N)
    Q = ly["Q"]

    # scatter fp16-cast edge features into the padded lane layout
    x16 = np.clip(x, -60000.0, 60000.0).astype(np.float16)
    perm = np.full((NLANES, Q), E, dtype=np.int64)
    perm[ly["elane"], ly["slot"]] = ly["order"]
    x_aug = np.concatenate(
        [x16, np.full((1, D), FILL, dtype=np.float16)], axis=0)
    g = x_aug[perm]                                   # (32, Q, D)
    g = g.reshape(C, L, Q, D).transpose(0, 1, 3, 2)   # (C, L, D, Q)
    xt_all = np.ascontiguousarray(g.reshape(C, P, Q))

    nc = _build_graph(ly)
    in_maps = [{"xt": xt_all[c]} for c in range(C)]
    res = run_bass_kernel_spmd(nc, in_maps, core_ids=list(range(C)))

    v = np.stack([res.results[c]["out"] for c in range(C)])   # (C, P, NN) f16
    out = np.zeros((N, D), dtype=np.float32)
    nzi = np.flatnonzero(ly["nz"])
    gl = ly["node_lane"][nzi].astype(np.int64)
    core, lane = gl // L, gl % L
    colv = ly["node_col"][nzi]
    rows = (lane * D)[:, None] + np.arange(D)[None, :]
    out[nzi] = v[core[:, None], rows, colv[:, None]].astype(np.float32)
    return out


# revision 5
# speedup vs baseline: 2.3071x; 1.1547x over previous
"""Distributed scatter-max (segment max over edge targets) on 8 TRN2 NeuronCores.

Strategy (segment-parallel scatter per the sharding hint):
  * Host shuffles edges: sort by target node; group nodes by exact degree and
    deal each degree class round-robin into 32 streams (8 cores x 4 lanes), so
    every stream has the IDENTICAL sequence of (window, count) chunks -> one
    SPMD graph serves all cores. Each node's edges are contiguous in its
    stream; streams are laid out feature-major (D=32 dims on SBUF partitions,
    edge slots along the free axis), cast to fp16 (max is order-exact; fp16
    rounding keeps rel err ~3e-4, far under the 2e-2 gate).
  * Device (same graph on all 8 cores) streams its (128, Q) slab tile by tile
    on the sync HWDGE queue and runs one windowed max tensor_reduce per degree
    chunk (window = exact degree -> zero compute waste), writing per-node
    columns; per-tile output slices leave on the scalar HWDGE queue so output
    never blocks the input stream.
  * Host gathers each node's column, casts back to f32; empty nodes stay 0.
"""
import sys

import numpy as np

try:
    import concourse.bacc as bacc
except ImportError:
    sys.path.insert(0, "/opt/trn_rl_repo")
    import concourse.bacc as bacc

import concourse.tile as tile
from concourse import mybir
from concourse.bass_utils import run_bass_kernel_spmd

C = 8            # cores
L = 4            # lanes per core (L * D = 128 partitions)
D = 32           # feature dim
P = 128
NLANES = C * L
T_MAX = 4096     # max edge slots per tile
FILL = -60000.0  # pad value (below any fp16 data value we handle)
DT = mybir.dt.float16

_graph_cache = {}


def _plan(tgt, N):
    """Host-side shuffle plan. Returns layout dict (see kernel())."""
    E = tgt.shape[0]
    deg = np.bincount(tgt, minlength=N).astype(np.int64)

    nz = np.flatnonzero(deg > 0)
    order_by_deg = nz[np.argsort(deg[nz], kind="stable")]
    degs_sorted = deg[order_by_deg]
    uniq, counts = np.unique(degs_sorted, return_counts=True)

    node_lane = np.full(N, -1, dtype=np.int32)
    node_rank = np.full(N, -1, dtype=np.int64)   # per-lane sequence index
    classes = []          # (window s, nodes per lane)
    seq_len = 0
    base = 0
    for s, cnt in zip(uniq, counts):
        ids = order_by_deg[base:base + cnt]
        base += cnt
        npl = (cnt + NLANES - 1) // NLANES
        node_lane[ids] = np.arange(cnt, dtype=np.int32) % NLANES
        node_rank[ids] = seq_len + np.arange(cnt, dtype=np.int64) // NLANES
        classes.append((int(s), int(npl)))
        seq_len += npl

    needed0 = sum(s * npl for s, npl in classes)
    # a node's window must fit inside one tile
    max_s = max(s for s, _ in classes)
    t_max = max(T_MAX, ((max_s + 63) // 64) * 64)
    assert max_s <= 24576, f"node degree {max_s} exceeds supported maximum"

    def walk(bounds):
        chunks = []           # (tile, off, n, s, col0)
        node_pos_seq = np.empty(seq_len, dtype=np.int64)
        ti = pos = col = seq_base = 0
        for s, npl in classes:
            remaining = npl
            while remaining > 0:
                if ti >= len(bounds):
                    return None
                start, size = bounds[ti]
                space = start + size - pos
                fit = min(remaining, space // s)
                if fit == 0:
                    pos = start + size
                    ti += 1
                    continue
                chunks.append((ti, int(pos - start), int(fit), int(s), int(col)))
                idx0 = seq_base + (npl - remaining)
                node_pos_seq[idx0:idx0 + fit] = pos + np.arange(fit, dtype=np.int64) * s
                pos += fit * s
                col += fit
                remaining -= fit
            seq_base += npl
        return chunks, node_pos_seq, col

    slack = 0
    while True:
        rem = needed0 + slack
        sizes = []
        # small head tiles so the vector engine starts early
        for t in (1024, 3072):
            if rem > 2 * t_max and t > max_s:
                sizes.append(t)
                rem -= t
        while rem > t_max:
            sizes.append(t_max)
            rem -= t_max
        sizes.append(((rem + 63) // 64) * 64)
        bounds = []
        acc = 0
        for t in sizes:
            bounds.append((acc, t))
            acc += t
        r = walk(bounds)
        if r is not None:
            break
        slack += 256
    chunks, node_pos_seq, NN = r

    node_pos = np.zeros(N, dtype=np.int64)
    node_col = np.full(N, -1, dtype=np.int64)
    m = node_rank >= 0
    node_pos[m] = node_pos_seq[node_rank[m]]
    node_col[m] = node_rank[m]           # cols assigned in walk (= seq) order

    order = np.argsort(tgt, kind="stable")
    sorted_tgt = tgt[order]
    starts = np.searchsorted(sorted_tgt, np.arange(N), side="left")
    rank = np.arange(E, dtype=np.int64) - starts[sorted_tgt]
    slot = node_pos[sorted_tgt] + rank
    elane = node_lane[sorted_tgt]

    return dict(chunks=chunks, tile_bounds=bounds, Q=int(acc), NN=int(NN),
                node_lane=node_lane, node_col=node_col, nz=deg > 0,
                order=order, slot=slot, elane=elane)


def _build_graph(ly):
    key = (ly["Q"], ly["NN"], tuple(ly["tile_bounds"]), tuple(ly["chunks"]))
    if key in _graph_cache:
        return _graph_cache[key]
    Q, NN = ly["Q"], ly["NN"]
    tile_bounds = ly["tile_bounds"]
    NT = len(tile_bounds)
    nc = bacc.Bacc()
    x_ext = nc.declare_dram_parameter("xt", [P, Q], DT, isOutput=False)
    out_ext = nc.declare_dram_parameter("out", [P, NN], DT, isOutput=True)
    by_tile = [[] for _ in range(NT)]
    for (ti, off, n, s, col0) in ly["chunks"]:
        by_tile[ti].append((off, n, s, col0))
    tile_cols = []
    for i in range(NT):
        cs = by_tile[i]
        c0 = min(c[3] for c in cs) if cs else 0
        c1 = max(c[3] + c[1] for c in cs) if cs else 0
        tile_cols.append((c0, c1))

    with tile.TileContext(nc) as tc:
        with tc.tile_pool(name="x", bufs=3) as xp, \
             tc.tile_pool(name="o", bufs=3) as opool:
            for i in range(NT):
                start, size = tile_bounds[i]
                xt = xp.tile([P, size], DT, tag="xt")
                nc.sync.dma_start(out=xt[:], in_=x_ext[:, start:start + size])
                c0, c1 = tile_cols[i]
                if c1 <= c0:
                    continue
                ot = opool.tile([P, c1 - c0], DT, tag="ot")
                for (off, n, s, col0) in by_tile[i]:
                    if s == 1:
                        nc.vector.tensor_copy(ot[:, col0 - c0:col0 - c0 + n],
                                              xt[:, off:off + n])
                    else:
                        nc.vector.tensor_reduce(
                            out=ot[:, col0 - c0:col0 - c0 + n],
                            in_=xt[:, off:off + n * s].rearrange(
                                "p (n s) -> p n s", s=s),
                            axis=mybir.AxisListType.X,
                            op=mybir.AluOpType.max,
                        )
                nc.scalar.dma_start(out=out_ext[:, c0:c1], in_=ot[:])
    nc.finalize()
    _graph_cache[key] = nc
    return nc


def kernel(source_node_representation_with_coefficient, edge_index, num_nodes):
    x = np.asarray(source_node_representation_with_coefficient, dtype=np.float32)
    tgt = np.asarray(edge_index)[1].astype(np.int64)
    N = int(num_nodes)
    E, d = x.shape
    assert d == D, f"kernel hardcodes D={D}, got {d}"
    if E == 0 or N == 0:
        return np.zeros((N, D), dtype=np.float32)

    ly = _plan(tgt, N)
    Q = ly["Q"]

    # scatter fp16-cast edge features into the padded lane layout
    x16 = np.clip(x, -60000.0, 60000.0).astype(np.float16)
    perm = np.full((NLANES, Q), E, dtype=np.int64)
    perm[ly["elane"], ly["slot"]] = ly["order"]
    x_aug = np.concatenate(
        [x16, np.full((1, D), FILL, dtype=np.float16)], axis=0)
    g = x_aug[perm]                                   # (32, Q, D)
    g = g.reshape(C, L, Q, D).transpose(0, 1, 3, 2)   # (C, L, D, Q)
    xt_all = np.ascontiguousarray(g.reshape(C, P, Q))

    nc = _build_graph(ly)
    in_maps = [{"xt": xt_all[c]} for c in range(C)]
    res = run_bass_kernel_spmd(nc, in_maps, core_ids=list(range(C)))

    v = np.stack([res.results[c]["out"] for c in range(C)])   # (C, P, NN) f16
    out = np.zeros((N, D), dtype=np.float32)
    nzi = np.flatnonzero(ly["nz"])
    gl = ly["node_lane"][nzi].astype(np.int64)
    core, lane = gl // L, gl % L
    colv = ly["node_col"][nzi]
    rows = (lane * D)[:, None] + np.arange(D)[None, :]
    out[nzi] = v[core[:, None], rows, colv[:, None]].astype(np.float32)
    return out
